# revision 8
# baseline (speedup 1.0000x reference)
"""Sparse (chunked-causal | bidirectional-block) GQA attention on 8 trn2 cores.

Full inputs in, full output out. Sharding: core j handles batch b = j // 4 and
kv-heads {2*(j%4), 2*(j%4)+1} (= query heads 4*(j%4) .. 4*(j%4)+3).

Host prep (per core, numpy): cast to fp16 and pre-transpose so the device
does zero layout work:
  - qT [4, 128(d), 2048(s)], kT [2, 128(d), 2048(s)]  (big-packet DMAs)
  - vA [2, 16, 128, 132]: V with a ones column at d=128 (softmax denominators
    fall out of the PV matmul for free)
  - masks: additive f16 bias blocks {0, -30000} for partial 128x128 tiles,
    partition = kv
  - ident [128, 128] f16 identity

Per-core bass kernel (fp16 on-chip, fp32 PSUM):
  - S^T[kv, q] per 128-kv-tile via PE matmul (lhsT = K^T tile, rhs = Q^T
    cols), packed into <=2-bank PSUM rounds (bufs=2 -> QK of round r+1
    overlaps exp of round r).
  - Partial blocks get their mask added IN PSUM via a PE matmul with
    identity weights (PSUM += I^T @ bias = bias), start=False. This keeps
    the DVE off the critical path entirely.
  - One ACT exp (scale=1/sqrt(D)) per round -> E (fp16, SBUF).
  - PV: per q-subtile accumulate matmuls lhsT=E-slice, rhs=vA tile,
    own PSUM bank region; ones column gives denominators.
  - Normalize per group: one DVE reciprocal [128,4] + one broadcast mul.
  - Loads are split across scalar/sync/gpsimd DGE queues with the head-0
    group-0 working set prioritized so compute starts ~3us in.

The block schedule is computed on the host from the actual mask + chunk_size
as the union over both batch elements (SPMD across cores); mask bias data
stays exact per core.
"""

import math

import numpy as np

import concourse.bass as bass
import concourse.mybir as mybir
import concourse.tile as tile
from concourse import bacc
from concourse.bass import _add_dep_helper
from concourse.bass_utils import run_bass_kernel_spmd

B, S, HQ, HKV, D = 2, 2048, 16, 8, 128
TS = 128                  # block tile size (partitions)
NT = S // TS              # 16 q/kv tiles
GROUP_SUBTILES = 4        # q-subtiles per group (512 q rows)
N_GROUPS = NT // GROUP_SUBTILES
ROUND_COLS_CAP = 1024     # 2 fp32 PSUM banks per round
BANK_COLS = 512           # fp32 cols per PSUM bank
N_CORES = 8
PAIRS_PER_CORE = 2        # kv heads per core
HEADS_PER_CORE = 4        # query heads per core
MASK_BIAS = -30000.0      # exp(scale*(x+MASK_BIAS)) == 0 in fp16

F16 = mybir.dt.float16
F32 = mybir.dt.float32


# ---------------------------------------------------------------- host masks

def _segment_ids(m):
    """[B, S] 0/1 -> contiguous-run segment ids (0 = not in a run)."""
    mm = m.astype(np.int64)
    padded = np.pad(mm, ((0, 0), (1, 0)))
    boundary = padded[:, 1:] > padded[:, :-1]
    return mm * np.cumsum(boundary, axis=1)


def _allowed_T(bidirectional_mask, chunk):
    """Per-batch allowed mask, transposed: [B, S(kv), S(q)] bool."""
    seg = _segment_ids(np.asarray(bidirectional_mask))
    r = np.arange(S)
    chunk_ok = (r[:, None] // chunk == r[None, :] // chunk) & (r[:, None] >= r[None, :])
    out = np.zeros((B, S, S), dtype=bool)
    for b in range(B):
        bid = (seg[b][:, None] == seg[b][None, :]) & (seg[b][:, None] > 0)
        out[b] = (chunk_ok | bid).T
    return out


class Schedule:
    """Static (union-over-batch) block schedule, shared by all 8 cores."""

    def __init__(self, allowed_T):
        blocks = allowed_T.reshape(B, NT, TS, NT, TS)
        b_any = blocks.any(axis=(2, 4))   # [B, t, s]
        b_all = blocks.all(axis=(2, 4))
        self.u_any = b_any.any(axis=0)    # [t, s]
        self.u_all = b_all.all(axis=0)
        self.partial = self.u_any & ~self.u_all

        self.mask_blocks = []             # list of (t, s) in fixed order
        mask_idx = {}

        # groups[g] = list of rounds; round = dict with fields:
        #   cols: total packed columns
        #   qk: list of (t, coff, q0, n)            matmul pieces
        #   masks: list of (e_off, midx, nblk)      merged PE bias pieces
        #   pv: {s_local: [(t, e_off), ...]}        accumulation lists
        self.groups = []
        self.group_tmax = []              # max kv-tile index touched, per group
        self.group_mask_hi = []           # mask_blocks prefix length after group
        for g in range(N_GROUPS):
            s0, s1 = g * GROUP_SUBTILES, (g + 1) * GROUP_SUBTILES
            t_entries = []
            for t in range(NT):
                ss = [s for s in range(s0, s1) if self.u_any[t, s]]
                if not ss:
                    continue
                lo, hi = min(ss), max(ss) + 1
                t_entries.append((t, lo, hi))
            self.group_tmax.append(max(t for t, _, _ in t_entries))

            rounds = []
            cur = None
            for (t, lo, hi) in t_entries:
                ncols = (hi - lo) * TS
                if cur is None or cur["cols"] + ncols > ROUND_COLS_CAP:
                    cur = {"cols": 0, "qk": [], "raw_masks": [],
                           "pv": {sl: [] for sl in range(GROUP_SUBTILES)}}
                    rounds.append(cur)
                toff = cur["cols"]
                # split matmul pieces at PSUM bank boundaries
                q0 = lo * TS
                off = toff
                rem = ncols
                while rem > 0:
                    n = min(BANK_COLS - off % BANK_COLS, rem)
                    cur["qk"].append((t, off, q0, n))
                    off += n
                    q0 += n
                    rem -= n
                for s in range(lo, hi):
                    if not self.u_any[t, s]:
                        continue
                    e_off = toff + (s - lo) * TS
                    if self.partial[t, s]:
                        if (t, s) not in mask_idx:
                            mask_idx[(t, s)] = len(self.mask_blocks)
                            self.mask_blocks.append((t, s))
                        cur["raw_masks"].append((e_off, mask_idx[(t, s)]))
                    cur["pv"][s - s0].append((t, e_off))
                cur["cols"] += ncols

            # merge adjacent bias pieces (contiguous e cols + mask idxs),
            # then split at QK-piece boundaries (a start=True QK matmul
            # must never land on columns whose bias is already applied)
            for rnd in rounds:
                cuts = sorted({coff for (_, coff, _, _) in rnd["qk"]})

                def piece_end(off):
                    ends = [c for c in cuts if c > off]
                    return min(ends) if ends else ROUND_COLS_CAP

                merged = []
                for (e_off, midx) in sorted(rnd.pop("raw_masks")):
                    if (merged and merged[-1][0] + merged[-1][2] * TS == e_off
                            and merged[-1][1] + merged[-1][2] == midx):
                        merged[-1][2] += 1
                    else:
                        merged.append([e_off, midx, 1])
                pieces = []
                for (e_off, midx, nblk) in merged:
                    off, mi, rem = e_off, midx * TS, nblk * TS
                    while rem > 0:
                        n = min(piece_end(off) - off, rem)
                        pieces.append((off, mi, n))
                        off += n
                        mi += n
                        rem -= n
                rnd["masks"] = pieces
            self.groups.append(rounds)
            self.group_mask_hi.append(len(self.mask_blocks))

        self.n_masks = len(self.mask_blocks)

    def mask_data(self, allowed_T_b):
        """[TS, n_masks, TS] fp16 additive-bias blocks for one batch."""
        out = np.zeros((TS, max(self.n_masks, 1), TS), dtype=np.float16)
        for i, (t, s) in enumerate(self.mask_blocks):
            blk = allowed_T_b[t * TS:(t + 1) * TS, s * TS:(s + 1) * TS]
            out[:, i, :] = np.where(blk, 0.0, MASK_BIAS)
        return out

    def key(self):
        return (self.u_any.tobytes(), self.u_all.tobytes())


# ------------------------------------------------------------- kernel build

def _broadcast_free(ap, n):
    """Append a 0-step free dim of size n to an AP (read-broadcast)."""
    return bass.AP(tensor=ap.tensor, offset=ap.offset, ap=[*ap.ap, [0, n]])


def _build_body(nc, tc, sched: Schedule, tensors, safe_pv=False):
    qT_in, kT_in, vA_in, m_in, id_in, o_out = tensors
    scale = 1.0 / math.sqrt(D)
    ctxs = []
    pv_first_mms = []   # (first_inst_name, [other_inst_names]) per PSUM bank

    def pool(*a, **kw):
        p = tc.tile_pool(*a, **kw)
        ctxs.append(p)
        return p.__enter__()

    consts = pool(name="consts", bufs=1)
    epool = pool(name="epool", bufs=5)
    outp = pool(name="outp", bufs=3)
    small = pool(name="small", bufs=4)
    stp = pool(name="st_psum", bufs=2, space="PSUM")
    pvp = pool(name="pv_psum", bufs=1 if safe_pv else 2, space="PSUM")

    nmask = max(sched.n_masks, 1)
    kt = consts.tile([TS, PAIRS_PER_CORE, S], F16)
    qt = consts.tile([TS, HEADS_PER_CORE, S], F16)
    vA = consts.tile([TS, PAIRS_PER_CORE, NT, D + 4], F16)
    mask_sb = consts.tile([TS, nmask, TS], F16)
    ident = consts.tile([TS, TS], F16)

    # Priority DMA issue: head-0 group-0 working set first, split across
    # the three DGE queues (scalar HWDGE, gpsimd SWDGE, sync HWDGE).
    t0hi = min(max(sched.group_tmax[0] + 1, GROUP_SUBTILES), NT)
    c0 = t0hi * TS
    m0 = max(sched.group_mask_hi[0], 1)

    # scalar: pair-0 K^T and head-0 Q^T, group-0 columns first
    nc.scalar.dma_start(out=kt[:, 0, 0:c0], in_=kT_in[0, :, 0:c0])
    nc.scalar.dma_start(out=qt[:, 0, 0:512], in_=qT_in[0, :, 0:512])
    nc.scalar.dma_start(out=kt[:, 0, c0:S], in_=kT_in[0, :, c0:S])
    nc.scalar.dma_start(out=qt[:, 0, 512:S], in_=qT_in[0, :, 512:S])

    # gpsimd: identity, group-0 masks + V, then the rest
    nc.gpsimd.dma_start(out=ident, in_=id_in[:, :])
    nc.gpsimd.dma_start(out=mask_sb[:, 0:m0, :], in_=m_in[:, 0:m0, :])
    nc.gpsimd.dma_start(
        out=vA[:, 0, 0:t0hi, 0:D + 1],
        in_=vA_in[0, 0:t0hi, :, 0:D + 1].rearrange("t p d -> p t d"),
    )
    if m0 < nmask:
        nc.gpsimd.dma_start(out=mask_sb[:, m0:nmask, :], in_=m_in[:, m0:nmask, :])
    nc.gpsimd.dma_start(
        out=vA[:, 0, t0hi:NT, 0:D + 1],
        in_=vA_in[0, t0hi:NT, :, 0:D + 1].rearrange("t p d -> p t d"),
    )
    nc.gpsimd.dma_start(
        out=vA[:, 1, :, 0:D + 1],
        in_=vA_in[1, :, :, 0:D + 1].rearrange("t p d -> p t d"),
    )

    # sync: remaining heads' Q^T / pair-1 K^T (in need order), then outputs
    nc.sync.dma_start(out=qt[:, 1, :], in_=qT_in[1, :, :])
    nc.sync.dma_start(out=kt[:, 1, :], in_=kT_in[1, :, :])
    nc.sync.dma_start(out=qt[:, 2, :], in_=qT_in[2, :, :])
    nc.sync.dma_start(out=qt[:, 3, :], in_=qT_in[3, :, :])

    # flatten all (head, group, round) work items for software-pipelined
    # emission: PV/normalize lag a few rounds behind QK/exp so the PE
    # stream never waits on exp of the round it just produced
    nbank = GROUP_SUBTILES if safe_pv else 2
    per = 1 if safe_pv else 2
    work = []
    for pair in range(PAIRS_PER_CORE):
        for g_head in range(2):
            head = 2 * pair + g_head
            for g in range(N_GROUPS):
                for ri, rnd in enumerate(sched.groups[g]):
                    work.append({
                        "head": head, "pair": pair, "g": g, "rnd": rnd,
                        "first": ri == 0,
                        "last": ri == len(sched.groups[g]) - 1,
                    })

    group_state = {}

    def emit_front(w):
        st = stp.tile([TS, ROUND_COLS_CAP], F32, tag="st")
        pair, head = w["pair"], w["head"]
        # bias pieces grouped under the QK piece whose region they land in
        bias_of = {}   # qk piece index -> [(e_off, moff, n), ...]
        for (e_off, moff, n) in w["rnd"]["masks"]:
            for qi, (t, coff, q0, qn) in enumerate(w["rnd"]["qk"]):
                if coff <= e_off < coff + qn:
                    bias_of.setdefault(qi, []).append((e_off, moff, n))
                    break
        for qi, (t, coff, q0, n) in enumerate(w["rnd"]["qk"]):
            biases = bias_of.get(qi, [])
            mm = nc.tensor.matmul(
                st[:, coff:coff + n],
                lhsT=kt[:, pair, t * TS:(t + 1) * TS],
                rhs=qt[:, head, q0:q0 + n],
                start=True, stop=not biases,
            )
            prev = mm.ins
            for bi, (e_off, moff, bn) in enumerate(biases):
                bm = nc.tensor.matmul(
                    st[:, e_off:e_off + bn],
                    lhsT=ident,
                    rhs=mask_sb[:, moff // TS:(moff + bn) // TS, :],
                    start=False, stop=bi == len(biases) - 1,
                )
                # the accumulate must run after the start=True QK write
                _add_dep_helper(bm.ins, prev, sync=True,
                                reason="psum bias accumulate after QK start")
                prev = bm.ins
        e = epool.tile([TS, ROUND_COLS_CAP], F16, tag="e")
        nc.scalar.activation(
            e[:, 0:w["rnd"]["cols"]], st[:, 0:w["rnd"]["cols"]],
            mybir.ActivationFunctionType.Exp, scale=scale,
        )
        w["e"] = e

    def emit_back(w):
        g, head = w["g"], w["head"]
        if w["first"]:
            gs = {
                "pv": pvp.tile([TS, nbank, per, BANK_COLS // per], F32,
                               name=f"pv_{head}_{g}", tag="pv"),
                "bank_first": [None] * nbank,
                "bank_mms": [[] for _ in range(nbank)],
                "bank_total": [0] * nbank,
                "bank_done": [0] * nbank,
            }
            for r in sched.groups[g]:
                for sl in range(GROUP_SUBTILES):
                    gs["bank_total"][sl // per] += len(r["pv"][sl])
            group_state[(head, g)] = gs
        gs = group_state[(head, g)]
        pv, e = gs["pv"], w["e"]
        for sl in range(GROUP_SUBTILES):
            bk, sub = divmod(sl, per)
            for (t, e_off) in w["rnd"]["pv"][sl]:
                first = gs["bank_first"][bk] is None
                gs["bank_done"][bk] += 1
                mm = nc.tensor.matmul(
                    pv[:, bk, sub, 0:D + 1],
                    lhsT=e[:, e_off:e_off + TS],
                    rhs=vA[:, w["pair"], t, 0:D + 1],
                    start=first,
                    stop=gs["bank_done"][bk] == gs["bank_total"][bk],
                )
                # chain the bank's accumulation window: start first, stop
                # last, deterministically (the tile scheduler is otherwise
                # free to reorder disjoint-region matmuls in the bank)
                if gs.get("bank_prev", [None] * nbank)[bk] is not None:
                    _add_dep_helper(mm.ins, gs["bank_prev"][bk], sync=True,
                                    reason="psum bank accumulate chain")
                gs.setdefault("bank_prev", [None] * nbank)[bk] = mm.ins
                if first:
                    gs["bank_first"][bk] = mm.ins.name
                else:
                    gs["bank_mms"][bk].append(mm.ins.name)
        if not w["last"]:
            return
        pv_first_mms.extend(
            (f, o) for f, o in zip(gs["bank_first"], gs["bank_mms"])
            if f is not None)
        recip = small.tile([TS, nbank, per], F32, tag="recip")
        nc.vector.reciprocal(recip, pv[:, :, :, D])
        out_sb = outp.tile([TS, nbank, per, D], F16, tag="outsb")
        nc.vector.tensor_mul(out_sb, pv[:, :, :, 0:D],
                             _broadcast_free(recip, D))
        rows = GROUP_SUBTILES * TS
        nc.sync.dma_start(
            out=o_out[g * rows:(g + 1) * rows, head, :]
                .rearrange("(t p) d -> p t d", p=TS),
            in_=out_sb,
        )

    LAG = min(3, max(1, len(work) - 1))
    for i, w in enumerate(work):
        emit_front(w)
        if i >= LAG:
            emit_back(work[i - LAG])
    for w in work[len(work) - LAG:]:
        emit_back(w)

    for p in reversed(ctxs):
        p.__exit__(None, None, None)
    return pv_first_mms


def _verify_pv_order(nc, pv_first_mms):
    """Each PSUM bank's start=True matmul must precede its other matmuls in
    the final (scheduled) program order."""
    pos = {}
    i = 0
    for bb in nc.m.functions[0].blocks:
        for ins in bb.instructions:
            pos[ins.name] = i
            i += 1
    for first, others in pv_first_mms:
        p0 = pos.get(first)
        if p0 is None:
            return False
        for o in others:
            po = pos.get(o)
            if po is None or po < p0:
                return False
    return True


def _build_kernel(sched: Schedule, safe_pv: bool = False):
    nc = bacc.Bacc("TRN2", target_bir_lowering=False, debug=False,
                   num_devices=N_CORES, name="sparse_attn")

    qT_in = nc.dram_tensor("qT_sh", [HEADS_PER_CORE, TS, S], F16,
                           kind="ExternalInput")
    kT_in = nc.dram_tensor("kT_sh", [PAIRS_PER_CORE, TS, S], F16,
                           kind="ExternalInput")
    vA_in = nc.dram_tensor("vA_sh", [PAIRS_PER_CORE, NT, TS, D + 4], F16,
                           kind="ExternalInput")
    m_in = nc.dram_tensor("masks", [TS, max(sched.n_masks, 1), TS], F16,
                          kind="ExternalInput")
    id_in = nc.dram_tensor("ident", [TS, TS], F16, kind="ExternalInput")
    o_out = nc.dram_tensor("o_sh", [S, HEADS_PER_CORE, D], F16,
                           kind="ExternalOutput")
    tensors = (qT_in, kT_in, vA_in, m_in, id_in, o_out)

    with tile.TileContext(nc) as tc:
        pv_first_mms = _build_body(nc, tc, sched, tensors, safe_pv=safe_pv)

    nc.compile()
    if not safe_pv and not _verify_pv_order(nc, pv_first_mms):
        return _build_kernel(sched, safe_pv=True)
    return nc


# --------------------------------------------------------------- entry point

_CACHE = {}
_IDENT = np.eye(TS, dtype=np.float16)


def _get_kernel(sched: Schedule):
    key = sched.key()
    if key not in _CACHE:
        _CACHE[key] = _build_kernel(sched)
    return _CACHE[key]


def _shard_inputs(q, k, v, masks_f16):
    in_maps = []
    for core in range(N_CORES):
        b = core // 4
        m = core % 4
        qh = q[b, :, 4 * m:4 * m + 4, :]          # [S, 4, D] f32
        kh = k[b, :, 2 * m:2 * m + 2, :]          # [S, 2, D]
        vh = v[b, :, 2 * m:2 * m + 2, :]
        qT = np.ascontiguousarray(
            qh.transpose(1, 2, 0).astype(np.float16))   # [4, D, S]
        kT = np.ascontiguousarray(
            kh.transpose(1, 2, 0).astype(np.float16))   # [2, D, S]
        vA = np.zeros((PAIRS_PER_CORE, NT, TS, D + 4), dtype=np.float16)
        vA[:, :, :, 0:D] = vh.reshape(NT, TS, 2, D).transpose(2, 0, 1, 3)
        vA[:, :, :, D] = 1.0
        in_maps.append({
            "qT_sh": qT,
            "kT_sh": kT,
            "vA_sh": vA,
            "masks": masks_f16[b],
            "ident": _IDENT,
        })
    return in_maps


def kernel(q, k, v, bidirectional_mask, chunk_size):
    q = np.asarray(q, dtype=np.float32)
    k = np.asarray(k, dtype=np.float32)
    v = np.asarray(v, dtype=np.float32)
    chunk = int(np.asarray(chunk_size))

    allowed_T = _allowed_T(bidirectional_mask, chunk)
    sched = Schedule(allowed_T)
    nc = _get_kernel(sched)

    masks_f16 = [sched.mask_data(allowed_T[b]) for b in range(B)]
    in_maps = _shard_inputs(q, k, v, masks_f16)

    res = run_bass_kernel_spmd(nc, in_maps, list(range(N_CORES)))

    out = np.empty((B, S, HQ, D), dtype=np.float32)
    for core in range(N_CORES):
        b = core // 4
        m = core % 4
        out[b, :, 4 * m:4 * m + 4, :] = res.results[core]["o_sh"].astype(np.float32)
    return out
